# revision 1
# baseline (speedup 1.0000x reference)
"""Trainium2 Bass kernel for a 2-layer GCN (nn_CorrelationGNN).

Math (reference):
    src,dst = edges + self loops;  deg over dst;  dinv = deg^-1/2
    h1 = relu(S @ (x @ W0) + b0),  S = D^-1/2 (A+I) D^-1/2
    h2 = relu(S @ (h1 @ W1) + b1)
    out = h2 @ Wf + bf

Factorization used: S @ (h W) = dinv * Agg(dinv * h) @ W, where Agg is the
pure 0/1 adjacency gather-sum (S commutes with the feature matmul).

Distribution: destination nodes sharded across 8 cores (12500/core, padded
to 12544 = 128*98).  Ranks are degree-sorted; rank r -> (p=r%128, g=r//128),
table row within a core slice = p*98+g.  Gather source is an fp16 table
[100352, 128] (row = 32 feats + 96 zeros = 256B) assembled per core from an
AllGather of compact fp16 slices.  Edges are gathered with gpsimd dma_gather
(int16 idxs, 4 SWDGE queues, <=1024 idxs/inst) as 4 source-quarter streams;
per (quarter, g-column) the slot count K is the max over all cores so the
traced program is identical on every core (SPMD).
"""

import numpy as np

import concourse.bass as bass  # noqa: F401
import concourse.bacc as bacc
import concourse.mybir as mybir
from concourse.tile import TileContext
from concourse.bass_utils import run_bass_kernel_spmd

P = 128
N = 100000
F = 32
NPC = 12500          # real nodes per core
G = 98               # g-columns per core
NPCP = P * G         # padded nodes per core = 12544
NROWS = 8 * NPCP     # global table rows = 100352
QROWS = NROWS // 4   # 25088, int16-addressable quarter
QZREL = 12543        # guaranteed-zero pad row, same offset in every quarter
KCAP = 8             # slots per dma_gather inst (8*128 = 1024 idx cap)
FDT = mybir.dt.float32
HDT = mybir.dt.float16


def _build_plan_and_offsets(edge_index):
    src = np.asarray(edge_index[0], dtype=np.int64)
    dst = np.asarray(edge_index[1], dtype=np.int64)
    loops = np.arange(N, dtype=np.int64)
    src = np.concatenate([src, loops])
    dst = np.concatenate([dst, loops])

    deg = np.bincount(dst, minlength=N).astype(np.float64)
    dinv = (1.0 / np.sqrt(deg)).astype(np.float32)

    node_core = np.arange(N) // NPC
    rank = np.empty(N, dtype=np.int64)
    perms = []
    for c in range(8):
        nodes = np.arange(c * NPC, (c + 1) * NPC)
        order = np.argsort(-deg[nodes], kind="stable")
        perm = nodes[order]
        perms.append(perm)
        rank[perm] = np.arange(NPC)
    trow = node_core * NPCP + (rank % P) * G + (rank // P)
    quarter = trow // QROWS
    qrel = (trow % QROWS).astype(np.int32)

    # per-core edges sorted by (dst rank, src quarter); self-loops handled
    # on-device as agg init = xd_own, so drop them from the token streams
    noloop = src != dst
    srcn, dstn = src[noloop], dst[noloop]
    edges = []
    cnt_rq = np.zeros((8, NPC * 4), dtype=np.int32)
    for c in range(8):
        m = (dstn // NPC) == c
        s, d = srcn[m], dstn[m]
        key = rank[d] * 4 + quarter[s]
        order = np.argsort(key, kind="stable")
        edges.append((qrel[s][order], key[order]))
        cnt_rq[c] = np.bincount(key, minlength=NPC * 4)

    crq = cnt_rq.reshape(8, NPC, 4)
    K = np.zeros((G, 4), dtype=np.int32)
    for g in range(G):
        K[g] = crq[:, g * P : (g + 1) * P, :].max(axis=(0, 1))

    # shared instruction plan: (q, g, k0, kc, col0)
    plan = []
    col = 0
    for q in range(4):
        for g in range(G):
            k0 = 0
            while k0 < int(K[g, q]):
                kc = min(KCAP, int(K[g, q]) - k0)
                plan.append((q, g, k0, kc, col))
                col += kc * P // 16
                k0 += kc
    totc = col

    # zero pad rows (pad ranks 12500.. of the two cores in each quarter have
    # dinv=0 so their table rows are always zero); spread pad tokens across
    # them to avoid HBM hot-spotting on a single row.
    pad_ranks = np.arange(NPC, NPCP)
    zrel = (pad_ranks % P) * G + (pad_ranks // P)  # within-slice rows
    zero_rows = np.concatenate([zrel, zrel + NPCP]).astype(np.int16)  # both cores

    # per-core offset tables [16, totc], idx i of inst at [i%16, col0+i//16]
    offs_all = []
    for c in range(8):
        qr, key = edges[c]
        cnt = np.bincount(key, minlength=NPC * 4)
        ptr = np.zeros(NPC * 4 + 1, dtype=np.int64)
        np.cumsum(cnt, out=ptr[1:])
        rngpad = np.random.default_rng(c)
        offs = zero_rows[rngpad.integers(0, len(zero_rows), size=totc * 16)].astype(
            np.int16
        )
        for (q, g, k0, kc, col0) in plan:
            base = col0 * 16
            lo = g * P
            nreal = min(P, NPC - lo)
            # vectorized over p
            ps = np.arange(nreal)
            rk = lo + ps
            a = ptr[rk * 4 + q]
            b = ptr[rk * 4 + q + 1]
            for kk in range(kc):
                k = k0 + kk
                sel = (a + k) < b
                pos = base + kk * P + ps[sel]
                offs[pos] = qr[(a + k)[sel]]
        offs_all.append(offs.reshape(totc, 16).T.copy())

    return plan, totc, dinv, perms, offs_all


def _build_program(plan, totc):
    nc = bacc.Bacc(
        "TRN2", target_bir_lowering=False, debug=False, num_devices=8,
        num_swdge_queues=4,
    )
    x_own = nc.declare_dram_parameter("x_own", [P, G * F], FDT, isOutput=False)
    dinv_own = nc.declare_dram_parameter("dinv_own", [P, G], FDT, isOutput=False)
    offs = nc.declare_dram_parameter("offs", [P, totc], mybir.dt.int16, isOutput=False)
    W0 = nc.declare_dram_parameter("W0", [F, F], FDT, isOutput=False)
    W1 = nc.declare_dram_parameter("W1", [F, F], FDT, isOutput=False)
    Wf = nc.declare_dram_parameter("Wf", [F, F], FDT, isOutput=False)
    b0 = nc.declare_dram_parameter("b0", [F, 1], FDT, isOutput=False)
    b1 = nc.declare_dram_parameter("b1", [F, 1], FDT, isOutput=False)
    bf = nc.declare_dram_parameter("bf", [F, 1], FDT, isOutput=False)
    out_own = nc.declare_dram_parameter("out_own", [P, G * F], FDT, isOutput=True)

    cc_in = nc.dram_tensor("cc_in", [NPCP, F], HDT)
    cc_out = nc.dram_tensor("cc_out", [NROWS, F], HDT, addr_space="Shared")
    table = nc.dram_tensor("table", [NROWS, 4 * F], HDT)

    from concourse.masks import make_identity

    with TileContext(nc) as tc:
        with (
            tc.tile_pool(name="persist", bufs=1) as pp,
            tc.tile_pool(name="gpool", bufs=10) as gp,
            tc.tile_pool(name="spool", bufs=4) as sp,
            tc.tile_pool(name="psum", bufs=2, space="PSUM") as psp,
        ):
            offs_t = pp.tile([P, totc], mybir.dt.int16)
            nc.sync.dma_start(out=offs_t[:], in_=offs[:])
            dinv_t = pp.tile([P, G], FDT)
            nc.sync.dma_start(out=dinv_t[:], in_=dinv_own[:])
            w0_t = pp.tile([F, F], FDT)
            nc.sync.dma_start(out=w0_t[:], in_=W0[:])
            w1_t = pp.tile([F, F], FDT)
            nc.sync.dma_start(out=w1_t[:], in_=W1[:])
            wf_t = pp.tile([F, F], FDT)
            nc.sync.dma_start(out=wf_t[:], in_=Wf[:])
            b0_t = pp.tile([F, 1], FDT)
            nc.sync.dma_start(out=b0_t[:], in_=b0[:])
            b1_t = pp.tile([F, 1], FDT)
            nc.sync.dma_start(out=b1_t[:], in_=b1[:])
            bf_t = pp.tile([F, 1], FDT)
            nc.sync.dma_start(out=bf_t[:], in_=bf[:])
            ident = pp.tile([P, P], FDT)
            make_identity(nc, ident[:])

            xcur = pp.tile([P, G * F], FDT, tag="xcur")
            nc.sync.dma_start(out=xcur[:], in_=x_own[:])
            agg = pp.tile([P, G * F], FDT, tag="agg")
            xd_own = pp.tile([P, G * F], HDT, tag="xdown")

            dinv_b = dinv_t[:].to_broadcast([P, G, F])

            def scale_to_table(src_tile, scope):
                with nc.named_scope(scope):
                    nc.vector.tensor_tensor(
                        out=xd_own[:].rearrange("p (g f) -> p g f", f=F),
                        in0=src_tile[:].rearrange("p (g f) -> p g f", f=F),
                        in1=dinv_b,
                        op=mybir.AluOpType.mult,
                    )
                    nc.sync.dma_start(out=cc_in[:], in_=xd_own[:])
                    nc.gpsimd.collective_compute(
                        "AllGather",
                        mybir.AluOpType.bypass,
                        replica_groups=[list(range(8))],
                        ins=[cc_in[:]],
                        outs=[cc_out[:]],
                    )
                    for qq in range(4):
                        nc.sync.dma_start(
                            out=table[qq * QROWS : (qq + 1) * QROWS, :F],
                            in_=cc_out[qq * QROWS : (qq + 1) * QROWS, :],
                        )

            def gather_layer(scope):
                with nc.named_scope(scope):
                    # self-loop contribution: agg starts at xd_own
                    nc.vector.tensor_copy(out=agg[:], in_=xd_own[:])
                    for (q, g, k0, kc, col0) in plan:
                        gt = gp.tile([P, KCAP, 4 * F], HDT, tag="g")
                        nc.gpsimd.dma_gather(
                            out_ap=gt[:, :kc, :],
                            in_ap=table[q * QROWS : (q + 1) * QROWS, :],
                            idxs_ap=offs_t[:, col0 : col0 + kc * P // 16],
                            num_idxs=kc * P,
                            num_idxs_reg=kc * P,
                            elem_size=4 * F,
                            queue_num=(q * G + g) % 4,
                        )
                        if kc == 1:
                            nc.vector.tensor_add(
                                out=agg[:, g * F : (g + 1) * F],
                                in0=agg[:, g * F : (g + 1) * F],
                                in1=gt[:, 0, :F],
                            )
                        else:
                            # fp16 pairs added into f32 (no fp16 accumulation)
                            h2 = kc // 2
                            h = (kc + 1) // 2
                            red = sp.tile([P, 4, F], FDT, tag="red")
                            nc.vector.tensor_add(
                                out=red[:, :h2, :],
                                in0=gt[:, 0 : 2 * h2 : 2, :F],
                                in1=gt[:, 1 : 2 * h2 : 2, :F],
                            )
                            if kc % 2:
                                nc.vector.tensor_copy(
                                    out=red[:, h2, :], in_=gt[:, kc - 1, :F]
                                )
                            if h == 1:
                                nc.vector.tensor_add(
                                    out=agg[:, g * F : (g + 1) * F],
                                    in0=agg[:, g * F : (g + 1) * F],
                                    in1=red[:, 0, :],
                                )
                            else:
                                red2 = sp.tile([P, F], FDT, tag="red2")
                                nc.vector.reduce_sum(
                                    out=red2[:],
                                    in_=red[:, :h, :].rearrange("p k f -> p f k"),
                                    axis=mybir.AxisListType.X,
                                )
                                nc.vector.tensor_add(
                                    out=agg[:, g * F : (g + 1) * F],
                                    in0=agg[:, g * F : (g + 1) * F],
                                    in1=red2[:],
                                )

            def layer_tail(W_t, bias_t, relu, dest, scope, W2_t=None, bias2_t=None):
                with nc.named_scope(scope):
                    nc.vector.tensor_tensor(
                        out=agg[:].rearrange("p (g f) -> p g f", f=F),
                        in0=agg[:].rearrange("p (g f) -> p g f", f=F),
                        in1=dinv_b,
                        op=mybir.AluOpType.mult,
                    )
                    for g in range(G):
                        ps1 = psp.tile([F, P], FDT, tag="ps1")
                        nc.tensor.matmul(
                            out=ps1[:], lhsT=agg[:, g * F : (g + 1) * F], rhs=ident[:],
                            start=True, stop=True,
                        )
                        s1 = sp.tile([F, P], FDT, tag="s1")
                        nc.vector.tensor_copy(out=s1[:], in_=ps1[:])
                        ps2 = psp.tile([F, P], FDT, tag="ps2")
                        nc.tensor.matmul(out=ps2[:], lhsT=W_t[:], rhs=s1[:], start=True, stop=True)
                        s2 = sp.tile([F, P], FDT, tag="s2")
                        if relu:
                            nc.scalar.activation(
                                out=s2[:], in_=ps2[:],
                                func=mybir.ActivationFunctionType.Relu,
                                bias=b0_t[:, :1] if bias_t is b0_t else bias_t[:, :1],
                                scale=1.0,
                            )
                        else:
                            nc.vector.tensor_scalar(
                                out=s2[:], in0=ps2[:], scalar1=bias_t[:, :1],
                                scalar2=None, op0=mybir.AluOpType.add,
                            )
                        if W2_t is not None:
                            ps3 = psp.tile([F, P], FDT, tag="ps3")
                            nc.tensor.matmul(out=ps3[:], lhsT=W2_t[:], rhs=s2[:], start=True, stop=True)
                            s2b = sp.tile([F, P], FDT, tag="s2b")
                            nc.vector.tensor_scalar(
                                out=s2b[:], in0=ps3[:], scalar1=bias2_t[:, :1],
                                scalar2=None, op0=mybir.AluOpType.add,
                            )
                            s2 = s2b
                        psb = psp.tile([P, F], FDT, tag="psb")
                        nc.tensor.matmul(
                            out=psb[:], lhsT=s2[:], rhs=ident[:F, :F], start=True, stop=True
                        )
                        nc.vector.tensor_copy(out=dest[:, g * F : (g + 1) * F], in_=psb[:])

            scale_to_table(xcur, "table0")
            gather_layer("gather0")
            layer_tail(w0_t, b0_t, relu=True, dest=xcur, scope="tail0")
            scale_to_table(xcur, "table1")
            gather_layer("gather1")
            outt = pp.tile([P, G * F], FDT, tag="outt")
            layer_tail(
                w1_t, b1_t, relu=True, dest=outt, scope="tail1", W2_t=wf_t, bias2_t=bf_t
            )
            nc.sync.dma_start(out=out_own[:], in_=outt[:])

    nc.compile()
    return nc


_CACHE = {}


def kernel(x, edge_index, W0, b0, W1, b1, Wf, bf):
    x = np.asarray(x, dtype=np.float32)
    edge_index = np.asarray(edge_index)
    plan, totc, dinv, perms, offs_all = _build_plan_and_offsets(edge_index)

    key = ("prog", totc, len(plan))
    if key not in _CACHE:
        _CACHE[key] = _build_program(plan, totc)
    nc = _CACHE[key]

    in_maps = []
    rr = np.arange(NPC)
    pp_, gg = rr % P, rr // P
    for c in range(8):
        perm = perms[c]
        xo = np.zeros((P, G, F), dtype=np.float32)
        dv = np.zeros((P, G), dtype=np.float32)
        xo[pp_, gg, :] = x[perm]
        dv[pp_, gg] = dinv[perm]
        in_maps.append(
            {
                "x_own": xo.reshape(P, G * F),
                "dinv_own": dv,
                "offs": np.tile(offs_all[c], (8, 1)).astype(np.int16),
                "W0": np.asarray(W0, np.float32),
                "W1": np.asarray(W1, np.float32),
                "Wf": np.asarray(Wf, np.float32),
                "b0": np.asarray(b0, np.float32).reshape(F, 1),
                "b1": np.asarray(b1, np.float32).reshape(F, 1),
                "bf": np.asarray(bf, np.float32).reshape(F, 1),
            }
        )

    res = run_bass_kernel_spmd(nc, in_maps, list(range(8)))
    kernel._last_results = res

    out = np.zeros((N, F), dtype=np.float32)
    for c in range(8):
        oo = res.results[c]["out_own"].reshape(P, G, F)
        out[perms[c]] = oo[pp_, gg, :]
    return out



# revision 5
# speedup vs baseline: 1.1294x; 1.1294x over previous
"""Trainium2 Bass kernel for a 2-layer GCN (nn_CorrelationGNN).

Math (reference):
    src,dst = edges + self loops;  deg over dst;  dinv = deg^-1/2
    h1 = relu(S @ (x @ W0) + b0),  S = D^-1/2 (A+I) D^-1/2
    h2 = relu(S @ (h1 @ W1) + b1)
    out = h2 @ Wf + bf

Factorization: S @ (h W) = dinv * Agg(dinv * h) @ W  (Agg = 0/1 adjacency
gather-sum; S commutes with the feature matmul).

Distribution: dst nodes sharded across 8 cores (12544 padded slots/core).
A node's table row is trow = core*12544 + p*98 + g (p = partition lane,
g = column).  The fp16 feature table [100352, 128] (row = 32 feats + 96
junk, 256B) is assembled per layer from an AllGather of compact fp16
slices, spread 64B->256B rows per quarter (25088 rows, int16-addressable).

Gathers run as a few large SWDGE dma_gather instructions per quarter
(~8K indices each) instead of hundreds of 1K-index ones: SWDGE cost is
dominated by a ~1-2us fixed overhead per instruction.  Slot padding is
minimized host-side by assigning nodes to (pair, column) groups with a
greedy vector bin-packing of their per-quarter in-edge count vectors
(K[g,q] = max over the column's 1024 lanes is what pads).

Per-column reduction: fp16 slot pairs added into f32, then a strided
reduce_sum, batched over runs of consecutive columns with equal K.

Layer tail: per 128-column chunk, transpose via identity matmul, multiply
by a block-diagonal(4x) weight matrix, bias+relu on the scalar engine,
transpose back.  25 chunks instead of 98 per-column pipelines.
"""

import numpy as np

import concourse.bass as bass  # noqa: F401
import concourse.bacc as bacc
import concourse.mybir as mybir
from concourse.tile import TileContext
from concourse.bass_utils import run_bass_kernel_spmd

P = 128
N = 100000
F = 32
G = 98               # columns per core
NPCP = P * G         # padded slots per core = 12544
NROWS = 8 * NPCP     # table rows = 100352
QROWS = NROWS // 4   # 25088 rows per quarter (int16-addressable)
NPAIR = 25000        # real nodes per core-pair (quarter)
import os as _os
CC = int(_os.environ.get("BK_CC", "64"))  # gather columns per instruction
USE_SCALAR_DMA = _os.environ.get("BK_SCALAR_DMA", "1") == "1"
RED_CAP = 32         # max R*ceil(K/2) per batched reduce
FDT = mybir.dt.float32
HDT = mybir.dt.float16


# ---------------------------------------------------------------- host plan

def _greedy_pack(c):
    """Assign len(c) nodes (rows of per-quarter count 4-vectors) to 98
    groups of <=256 minimizing sum_g sum_q max.  Returns assign array."""
    n = len(c)
    order = np.argsort(-(c.max(1) * 1000 + c.sum(1)), kind="stable")
    gmax = np.zeros((G, 4), dtype=np.int64)
    gcnt = np.zeros(G, dtype=np.int64)
    assign = np.empty(n, dtype=np.int64)
    for ii in order:
        v = c[ii]
        inc = np.maximum(0, v[None, :] - gmax).sum(axis=1).astype(np.float64)
        inc[gcnt >= 256] = 1e18
        best = inc.min()
        cand = np.where(inc == best)[0]
        g = cand[np.argmax(gcnt[cand])]
        assign[ii] = g
        gmax[g] = np.maximum(gmax[g], v)
        gcnt[g] += 1
    return assign


def _build_plan(edge_index):
    src = np.asarray(edge_index[0], dtype=np.int64)
    dst = np.asarray(edge_index[1], dtype=np.int64)
    noloop = src != dst
    src, dst = src[noloop], dst[noloop]

    indeg = np.bincount(dst, minlength=N)
    deg = indeg + 1  # self loop
    dinv = (1.0 / np.sqrt(deg.astype(np.float64))).astype(np.float32)

    # phase 1: round-robin by in-degree -> 4 balanced core-pairs (quarters)
    order = np.argsort(-indeg, kind="stable")
    pair = np.empty(N, dtype=np.int64)
    pair[order] = np.arange(N) % 4

    # per-node in-edge counts by src quarter
    cq = np.zeros((N, 4), dtype=np.int64)
    for q in range(4):
        np.add.at(cq[:, q], dst[pair[src] == q], 1)

    # phase 2: per-pair greedy packing into 98 column-groups of 256 lanes
    node_core = np.empty(N, dtype=np.int64)
    node_p = np.empty(N, dtype=np.int64)
    node_g = np.empty(N, dtype=np.int64)
    gmax_pair = np.zeros((4, G, 4), dtype=np.int64)
    empty_qrel = []  # per pair: qrels of empty (zero) slots
    for pr in range(4):
        nodes = np.where(pair == pr)[0]
        c = cq[nodes]
        assign = _greedy_pack(c)
        gm = np.zeros((G, 4), dtype=np.int64)
        np.maximum.at(gm, assign, c)
        # align across pairs: order groups by cost profile desc
        key = np.lexsort((-gm[:, 3], -gm[:, 2], -gm[:, 1], -gm[:, 0], -gm.sum(1)))
        rank_of = np.empty(G, dtype=np.int64)
        rank_of[key] = np.arange(G)
        gfinal = rank_of[assign]
        gmax_pair[pr] = gm[key]
        # lane position within group
        o = np.argsort(gfinal, kind="stable")
        lane = np.empty(len(nodes), dtype=np.int64)
        cnt = np.bincount(gfinal, minlength=G)
        starts = np.zeros(G + 1, dtype=np.int64)
        np.cumsum(cnt, out=starts[1:])
        lane[o] = np.arange(len(nodes)) - starts[gfinal[o]]
        node_core[nodes] = 2 * pr + (lane >= P)
        node_p[nodes] = lane % P
        node_g[nodes] = gfinal
        # empty slots of this pair
        occ = np.zeros((G, 256), dtype=bool)
        occ[gfinal, lane] = True
        eg, el = np.nonzero(~occ)
        etrow = (2 * pr + (el >= P)) * NPCP + (el % P) * G + eg
        empty_qrel.append((etrow - pr * QROWS).astype(np.int16))
        assert len(etrow) > 0

    trow = node_core * NPCP + node_p * G + node_g
    K = gmax_pair.max(axis=0)  # [G, 4]

    # column layout: quarter-major, g ascending
    totcols = int(K.sum())
    colstart = np.zeros((4, G), dtype=np.int64)
    qbase = np.zeros(5, dtype=np.int64)
    col = 0
    for q in range(4):
        qbase[q] = col
        for g in range(G):
            colstart[q, g] = col
            col += int(K[g, q])
    qbase[4] = col
    assert col == totcols

    # instruction chunks per quarter + per-chunk reduce runs
    chunks = []  # (q, col0, cc) in global column coords
    for q in range(4):
        c0 = qbase[q]
        while c0 < qbase[q + 1]:
            cc = min(CC, qbase[q + 1] - c0)
            chunks.append((int(q), int(c0), int(cc)))
            c0 += cc
    # interleave quarters round-robin
    per_q = [[ch for ch in chunks if ch[0] == q] for q in range(4)]
    chunks = []
    mx = max(len(x) for x in per_q)
    for i in range(mx):
        for q in range(4):
            if i < len(per_q[q]):
                chunks.append(per_q[q][i])

    # per-chunk groups: (g, off_in_chunk, klen), then merged equal-K runs
    col_g = np.zeros(totcols, dtype=np.int64)
    col_k = np.zeros(totcols, dtype=np.int64)
    for q in range(4):
        for g in range(G):
            s = colstart[q, g]
            k = int(K[g, q])
            col_g[s : s + k] = g
            col_k[s : s + k] = np.arange(k)
    chunk_runs = []
    for (q, c0, cc) in chunks:
        groups = []  # (g, off, klen)
        i = c0
        while i < c0 + cc:
            g = col_g[i]
            j = i
            while j < c0 + cc and col_g[j] == g:
                j += 1
            groups.append((int(g), int(i - c0), int(j - i)))
            i = j
        # merge consecutive g runs with equal klen == K[g,q] (full columns)
        runs = []
        i = 0
        while i < len(groups):
            g, off, kl = groups[i]
            r = 1
            while (
                i + r < len(groups)
                and groups[i + r][2] == kl
                and groups[i + r][0] == g + r
                and kl == int(K[groups[i + r][0], q])
                and kl == int(K[g, q])
                and (r + 1) * ((kl + 1) // 2) <= RED_CAP
            ):
                r += 1
            runs.append((g, off, kl, r))
            i += r
        chunk_runs.append(runs)

    plan = {
        "chunks": chunks,
        "chunk_runs": chunk_runs,
        "totcols": totcols,
        "K": K,
    }

    # ---------------- per-core gather offsets [16, totcols*8] int16
    rng = np.random.default_rng(7)
    col_quarter = np.zeros(totcols, dtype=np.int64)
    for q in range(4):
        col_quarter[qbase[q] : qbase[q + 1]] = q
    offs_all = []
    qrel_src = (trow - pair * QROWS).astype(np.int64)
    for core in range(8):
        offs = np.empty(totcols * P, dtype=np.int16)
        # pad default: random zero rows of the column's quarter
        for q in range(4):
            cols_q = np.nonzero(col_quarter == q)[0]
            npad = len(cols_q) * P
            pool = empty_qrel[q]
            offs_cols = pool[rng.integers(0, len(pool), size=npad)]
            view = offs.reshape(totcols, P)
            view[cols_q, :] = offs_cols.reshape(len(cols_q), P)
        m = node_core[dst] == core
        d, s = dst[m], src[m]
        gq = node_g[d] * 4 + pair[s]
        key = gq * P + node_p[d]
        o = np.argsort(key, kind="stable")
        ks = key[o]
        # cumcount within equal keys
        uniq, first = np.unique(ks, return_index=True)
        kk = np.arange(len(ks)) - first[np.searchsorted(uniq, ks)]
        colv = colstart[pair[s][o], node_g[d][o]] + kk
        pos = colv * P + node_p[d][o]
        offs[pos] = qrel_src[s][o].astype(np.int16)
        offs_all.append(np.tile(offs.reshape(totcols * 8, 16).T, (8, 1)))

    meta = {
        "node_core": node_core,
        "node_p": node_p,
        "node_g": node_g,
        "dinv": dinv,
    }
    return plan, offs_all, meta


# ---------------------------------------------------------------- program

def _build_program(plan):
    totcols = plan["totcols"]
    chunks = plan["chunks"]
    chunk_runs = plan["chunk_runs"]

    nc = bacc.Bacc(
        "TRN2", target_bir_lowering=False, debug=False, num_devices=8,
        num_swdge_queues=4,
    )
    x_own = nc.declare_dram_parameter("x_own", [P, G * F], FDT, isOutput=False)
    dinv_own = nc.declare_dram_parameter("dinv_own", [P, G], FDT, isOutput=False)
    offs = nc.declare_dram_parameter(
        "offs", [P, totcols * 8], mybir.dt.int16, isOutput=False
    )
    W0b = nc.declare_dram_parameter("W0b", [P, P], FDT, isOutput=False)
    W1b = nc.declare_dram_parameter("W1b", [P, P], FDT, isOutput=False)
    Wfb = nc.declare_dram_parameter("Wfb", [P, P], FDT, isOutput=False)
    b0b = nc.declare_dram_parameter("b0b", [P, 1], FDT, isOutput=False)
    b1b = nc.declare_dram_parameter("b1b", [P, 1], FDT, isOutput=False)
    bfb = nc.declare_dram_parameter("bfb", [P, 1], FDT, isOutput=False)
    out_own = nc.declare_dram_parameter("out_own", [P, G * F], FDT, isOutput=True)

    cc_in = nc.dram_tensor("cc_in", [NPCP, F], HDT)
    cc_out = nc.dram_tensor("cc_out", [NROWS, F], HDT, addr_space="Shared")
    tables = [
        nc.dram_tensor(f"table{q}", [QROWS, 4 * F], HDT) for q in range(4)
    ]

    from concourse.masks import make_identity

    with TileContext(nc) as tc:
        with (
            tc.tile_pool(name="persist", bufs=1) as pp,
            tc.tile_pool(name="gpool", bufs=3) as gp,
            tc.tile_pool(name="spool", bufs=4) as sp,
            tc.tile_pool(name="rpool", bufs=3) as rp,
            tc.tile_pool(name="psum", bufs=2, space="PSUM") as psp,
        ):
            offs_t = pp.tile([P, totcols * 8], mybir.dt.int16)
            nc.sync.dma_start(out=offs_t[:], in_=offs[:])
            dinv_t = pp.tile([P, G], FDT)
            nc.sync.dma_start(out=dinv_t[:], in_=dinv_own[:])
            w0_t = pp.tile([P, P], FDT)
            nc.sync.dma_start(out=w0_t[:], in_=W0b[:])
            w1_t = pp.tile([P, P], FDT)
            nc.sync.dma_start(out=w1_t[:], in_=W1b[:])
            wf_t = pp.tile([P, P], FDT)
            nc.sync.dma_start(out=wf_t[:], in_=Wfb[:])
            b0_t = pp.tile([P, 1], FDT)
            nc.sync.dma_start(out=b0_t[:], in_=b0b[:])
            b1_t = pp.tile([P, 1], FDT)
            nc.sync.dma_start(out=b1_t[:], in_=b1b[:])
            bf_t = pp.tile([P, 1], FDT)
            nc.sync.dma_start(out=bf_t[:], in_=bfb[:])
            ident = pp.tile([P, P], FDT)
            make_identity(nc, ident[:])

            xcur = pp.tile([P, G * F], FDT, tag="xcur")
            nc.sync.dma_start(out=xcur[:], in_=x_own[:])
            agg = pp.tile([P, G * F], FDT, tag="agg")
            xd_own = pp.tile([P, G * F], HDT, tag="xdown")
            outt = pp.tile([P, G * F], FDT, tag="outt")

            dinv_b = dinv_t[:].to_broadcast([P, G, F])

            def scale_to_table(src_tile, scope):
                with nc.named_scope(scope):
                    nc.vector.tensor_tensor(
                        out=xd_own[:].rearrange("p (g f) -> p g f", f=F),
                        in0=src_tile[:].rearrange("p (g f) -> p g f", f=F),
                        in1=dinv_b,
                        op=mybir.AluOpType.mult,
                    )
                    nc.sync.dma_start(out=cc_in[:], in_=xd_own[:])
                    nc.gpsimd.collective_compute(
                        "AllGather",
                        mybir.AluOpType.bypass,
                        replica_groups=[list(range(8))],
                        ins=[cc_in[:]],
                        outs=[cc_out[:]],
                    )
                    for q in range(4):
                        eng = nc.sync if (q % 2 == 0 or not USE_SCALAR_DMA) else nc.scalar
                        eng.dma_start(
                            out=tables[q][:, :F],
                            in_=cc_out[q * QROWS : (q + 1) * QROWS, :],
                        )

            def gather_layer(scope):
                with nc.named_scope(scope):
                    nc.vector.tensor_copy(out=agg[:], in_=xd_own[:])
                    for ci, (q, c0, cc) in enumerate(chunks):
                        gt = gp.tile([P, CC, 4 * F], HDT, tag="g")
                        nc.gpsimd.dma_gather(
                            out_ap=gt[:, :cc, :],
                            in_ap=tables[q][:, :],
                            idxs_ap=offs_t[:, c0 * 8 : (c0 + cc) * 8],
                            num_idxs=cc * P,
                            num_idxs_reg=cc * P,
                            elem_size=4 * F,
                            queue_num=q,
                        )
                        for (g, off, kl, r) in chunk_runs[ci]:
                            dst_ap = agg[:, g * F : (g + r) * F]
                            if kl == 1:
                                if r == 1:
                                    nc.vector.tensor_add(
                                        out=dst_ap,
                                        in0=dst_ap,
                                        in1=gt[:, off, :F],
                                    )
                                else:
                                    nc.vector.tensor_add(
                                        out=agg[:, g * F : (g + r) * F].rearrange(
                                            "p (r f) -> p r f", f=F
                                        ),
                                        in0=agg[:, g * F : (g + r) * F].rearrange(
                                            "p (r f) -> p r f", f=F
                                        ),
                                        in1=gt[:, off : off + r, :F],
                                    )
                                continue
                            h2 = kl // 2
                            h = (kl + 1) // 2
                            gtr = gt[:, off : off + r * kl, :].rearrange(
                                "p (r k) e -> p r k e", k=kl
                            )
                            red = rp.tile([P, RED_CAP, F], FDT, tag="red")
                            redv = red[:, : r * h, :].rearrange(
                                "p (r h) f -> p r h f", h=h
                            )
                            nc.vector.tensor_add(
                                out=redv[:, :, :h2, :],
                                in0=gtr[:, :, 0 : 2 * h2 : 2, :F],
                                in1=gtr[:, :, 1 : 2 * h2 : 2, :F],
                            )
                            if kl % 2:
                                nc.vector.tensor_copy(
                                    out=redv[:, :, h2, :],
                                    in_=gtr[:, :, kl - 1, :F],
                                )
                            if h == 1:
                                nc.vector.tensor_add(
                                    out=dst_ap.rearrange("p (r f) -> p r f", f=F),
                                    in0=dst_ap.rearrange("p (r f) -> p r f", f=F),
                                    in1=redv[:, :, 0, :],
                                )
                            else:
                                red2 = rp.tile([P, RED_CAP, F], FDT, tag="red2")
                                nc.vector.reduce_sum(
                                    out=red2[:, :r, :],
                                    in_=redv.rearrange("p r h f -> p r f h"),
                                    axis=mybir.AxisListType.X,
                                )
                                nc.vector.tensor_add(
                                    out=dst_ap.rearrange("p (r f) -> p r f", f=F),
                                    in0=dst_ap.rearrange("p (r f) -> p r f", f=F),
                                    in1=red2[:, :r, :],
                                )

            def layer_tail(W_t, bias_t, relu, dest, scope, W2_t=None, bias2_t=None):
                with nc.named_scope(scope):
                    nc.vector.tensor_tensor(
                        out=agg[:].rearrange("p (g f) -> p g f", f=F),
                        in0=agg[:].rearrange("p (g f) -> p g f", f=F),
                        in1=dinv_b,
                        op=mybir.AluOpType.mult,
                    )
                    ncols_total = G * F
                    j0 = 0
                    while j0 < ncols_total:
                        cols = min(P, ncols_total - j0)
                        psT = psp.tile([P, P], FDT, tag="psT")
                        nc.tensor.matmul(
                            out=psT[:cols, :],
                            lhsT=agg[:, j0 : j0 + cols],
                            rhs=ident[:],
                            start=True,
                            stop=True,
                        )
                        sT = sp.tile([P, P], FDT, tag="sT")
                        nc.vector.tensor_copy(out=sT[:cols, :], in_=psT[:cols, :])
                        ps2 = psp.tile([P, P], FDT, tag="ps2")
                        nc.tensor.matmul(
                            out=ps2[:cols, :],
                            lhsT=W_t[:cols, :cols],
                            rhs=sT[:cols, :],
                            start=True,
                            stop=True,
                        )
                        s2 = sp.tile([P, P], FDT, tag="s2")
                        if relu:
                            nc.scalar.activation(
                                out=s2[:cols, :],
                                in_=ps2[:cols, :],
                                func=mybir.ActivationFunctionType.Relu,
                                bias=bias_t[:cols, :1],
                                scale=1.0,
                            )
                        else:
                            nc.vector.tensor_scalar(
                                out=s2[:cols, :], in0=ps2[:cols, :],
                                scalar1=bias_t[:cols, :1],
                                scalar2=None, op0=mybir.AluOpType.add,
                            )
                        if W2_t is not None:
                            ps3 = psp.tile([P, P], FDT, tag="ps3")
                            nc.tensor.matmul(
                                out=ps3[:cols, :],
                                lhsT=W2_t[:cols, :cols],
                                rhs=s2[:cols, :],
                                start=True,
                                stop=True,
                            )
                            s3 = sp.tile([P, P], FDT, tag="s3")
                            nc.vector.tensor_scalar(
                                out=s3[:cols, :], in0=ps3[:cols, :],
                                scalar1=bias2_t[:cols, :1],
                                scalar2=None, op0=mybir.AluOpType.add,
                            )
                            s2 = s3
                        psb = psp.tile([P, P], FDT, tag="psb")
                        nc.tensor.matmul(
                            out=psb[:, :cols],
                            lhsT=s2[:cols, :],
                            rhs=ident[:cols, :cols],
                            start=True,
                            stop=True,
                        )
                        nc.vector.tensor_copy(
                            out=dest[:, j0 : j0 + cols], in_=psb[:, :cols]
                        )
                        j0 += cols

            scale_to_table(xcur, "table0")
            gather_layer("gather0")
            layer_tail(w0_t, b0_t, relu=True, dest=xcur, scope="tail0")
            scale_to_table(xcur, "table1")
            gather_layer("gather1")
            layer_tail(
                w1_t, b1_t, relu=True, dest=outt, scope="tail1",
                W2_t=wf_t, bias2_t=bf_t,
            )
            nc.sync.dma_start(out=out_own[:], in_=outt[:])

    nc.compile()
    return nc


_CACHE = {}


def kernel(x, edge_index, W0, b0, W1, b1, Wf, bf):
    x = np.asarray(x, dtype=np.float32)
    edge_index = np.asarray(edge_index)
    plan, offs_all, meta = _build_plan(edge_index)

    key = ("prog", plan["totcols"], len(plan["chunks"]))
    if key not in _CACHE:
        _CACHE[key] = _build_program(plan)
    nc = _CACHE[key]

    node_core = meta["node_core"]
    node_p = meta["node_p"]
    node_g = meta["node_g"]
    dinv = meta["dinv"]

    def blockdiag(W):
        W = np.asarray(W, np.float32)
        out = np.zeros((P, P), dtype=np.float32)
        for i in range(4):
            out[i * F : (i + 1) * F, i * F : (i + 1) * F] = W
        return out

    def blockbias(b):
        return np.tile(np.asarray(b, np.float32), 4).reshape(P, 1)

    in_maps = []
    for c in range(8):
        m = node_core == c
        nodes = np.nonzero(m)[0]
        pp_, gg = node_p[nodes], node_g[nodes]
        xo = np.zeros((P, G, F), dtype=np.float32)
        dv = np.zeros((P, G), dtype=np.float32)
        xo[pp_, gg, :] = x[nodes]
        dv[pp_, gg] = dinv[nodes]
        in_maps.append(
            {
                "x_own": xo.reshape(P, G * F),
                "dinv_own": dv,
                "offs": offs_all[c].astype(np.int16),
                "W0b": blockdiag(W0),
                "W1b": blockdiag(W1),
                "Wfb": blockdiag(Wf),
                "b0b": blockbias(b0),
                "b1b": blockbias(b1),
                "bfb": blockbias(bf),
            }
        )

    res = run_bass_kernel_spmd(nc, in_maps, list(range(8)))
    kernel._last_results = res

    out = np.zeros((N, F), dtype=np.float32)
    for c in range(8):
        m = node_core == c
        nodes = np.nonzero(m)[0]
        oo = res.results[c]["out_own"].reshape(P, G, F)
        out[nodes] = oo[node_p[nodes], node_g[nodes], :]
    return out
